# revision 6
# baseline (speedup 1.0000x reference)
"""Sparse attention (sparsemax) TRN2 kernel — 8 NeuronCores, SPMD.

Sharding: i-row parallel. Core c handles batch b=c//4, query rows
[(c%4)*512, (c%4+1)*512) for ALL 8 heads. k/v projections are computed
for the full sequence on every core of a batch (cheap, PE has slack);
q projection only for the core's rows. No collectives — each core
produces complete output rows.

Sparsemax per row solved via threshold iteration instead of sort:
tau* solves sum(relu(z - tau)) = 1 (piecewise-linear decreasing in
tau). Init tau0 = rowmax - 1 (a guaranteed lower bound of tau*), one
Newton step using the support count, then 6 secant iterations (exact
once the support stabilizes; converges to the fp16 noise floor).
All iterations operate on r = relu(z - tau0) — identical f values,
lets z be clamped at 0 so every ACT pass can use the Relu table.

Compute dtype fp16 (matmuls + sparsemax tensors), f32 PSUM/stats.
attn@v needs attn^T: PE transpose via identity (DMA transpose hits
per-instruction semaphore-wait caps under Tile).
"""
import sys

sys.path.insert(0, "/opt/trn_rl_repo")

import numpy as np
import concourse.bass as bass
import concourse.bacc as bacc
import concourse.mybir as mybir
import concourse.tile as tile
from concourse.bass_utils import run_bass_kernel_spmd

F32 = mybir.dt.float32
F16 = mybir.dt.float16
A = mybir.AluOpType
AF = mybir.ActivationFunctionType
AX = mybir.AxisListType

B, N, D = 2, 2048, 512
H, DH = 8, 64
SCALE = DH ** -0.5
ROWS = 512          # query rows per core
NT = ROWS // 128    # 4 row tiles per head on this core
NJB = N // 128      # 16 key blocks
NNB = N // 512      # 4 matmul N-chunks over keys
KC = D // 128       # 4 contraction chunks over model dim

# f-eval engine per iteration round: iter1 + 6 secant.  D=DVE, A=ACT, G=GPS
F_ENGINES = ["D", "A", "D", "G", "A", "D", "G"]
N_ITERS = len(F_ENGINES) - 1  # secant rounds after iter1


def build():
    nc = bacc.Bacc(None, target_bir_lowering=False)

    xT_ext = nc.declare_dram_parameter("xT", [D, N], F32, isOutput=False)
    xTq_ext = nc.declare_dram_parameter("xTq", [D, ROWS], F32, isOutput=False)
    wq_ext = nc.declare_dram_parameter("wq", [D, 512], F32, isOutput=False)
    wk_ext = nc.declare_dram_parameter("wk", [D, 512], F32, isOutput=False)
    wv_ext = nc.declare_dram_parameter("wv", [D, 512], F32, isOutput=False)
    wo_ext = nc.declare_dram_parameter("wo", [D, 512], F32, isOutput=False)
    bias_ext = nc.declare_dram_parameter("bias", [128, 512], F32, isOutput=False)
    idn_ext = nc.declare_dram_parameter("idn", [128, 128], F16, isOutput=False)
    out_ext = nc.declare_dram_parameter("out", [ROWS, 512], F32, isOutput=True)

    with tile.TileContext(nc) as tc:
        with (
            tc.tile_pool(name="persist", bufs=1) as pp,
            tc.tile_pool(name="stage", bufs=3) as stg,
            tc.tile_pool(name="zpool", bufs=3) as zp,
            tc.tile_pool(name="rpool", bufs=8) as rp,
            tc.tile_pool(name="scrpool", bufs=4) as scp,
            tc.tile_pool(name="statpool", bufs=2) as stp,
            tc.tile_pool(name="attnT", bufs=2) as atp,
            tc.tile_pool(name="outp", bufs=2) as op_,
        ):
            # ---------------- Phase A: loads + fp16 casts ----------------
            def load_cast(ext, kc, width, tag):
                s = stg.tile([128, width], F32, tag=f"st{width}")
                nc.gpsimd.dma_start(s[:], ext[kc * 128:(kc + 1) * 128, :])
                t = pp.tile([128, width], F16, tag=f"{tag}{kc}", name=f"{tag}{kc}")
                nc.vector.tensor_copy(t[:], s[:])
                return t

            xT16 = [load_cast(xT_ext, kc, N, "xT") for kc in range(KC)]
            xTq16 = [load_cast(xTq_ext, kc, ROWS, "xTq") for kc in range(KC)]
            wq16 = [load_cast(wq_ext, kc, 512, "wq") for kc in range(KC)]
            wk16 = [load_cast(wk_ext, kc, 512, "wk") for kc in range(KC)]
            wv16 = [load_cast(wv_ext, kc, 512, "wv") for kc in range(KC)]
            wo16 = [load_cast(wo_ext, kc, 512, "wo") for kc in range(KC)]
            bias32 = pp.tile([128, 512], F32, tag="bias")
            nc.gpsimd.dma_start(bias32[:], bias_ext[:])
            idn16 = pp.tile([128, 128], F16, tag="idn")
            nc.gpsimd.dma_start(idn16[:], idn_ext[:])

            # ---------------- Phase B: projections (PE) ----------------
            # kT[g]: [128(2 heads x 64), N] = (wk 2-head slice)^T @ x^T
            kT16 = [pp.tile([128, N], F16, tag=f"kT{g}", name=f"kT{g}") for g in range(4)]
            qT16 = [pp.tile([128, ROWS], F16, tag=f"qT{g}", name=f"qT{g}") for g in range(4)]
            v16 = pp.tile([128, NJB, 512], F16, tag="v16")
            aoT = [pp.tile([128, 512], F16, tag=f"aoT{g}", name=f"aoT{g}") for g in range(4)]

            with tc.tile_pool(name="psB", bufs=2, space=bass.MemorySpace.PSUM) as psB:
                for g in range(4):
                    ps = psB.tile([128, N], F32, tag="pskT", bufs=1)
                    for nb in range(NNB):
                        for kc in range(KC):
                            nc.tensor.matmul(
                                ps[:, nb * 512:(nb + 1) * 512],
                                wk16[kc][:, g * 128:(g + 1) * 128],
                                xT16[kc][:, nb * 512:(nb + 1) * 512],
                                start=(kc == 0), stop=(kc == KC - 1),
                            )
                    for nb in range(NNB):
                        nc.vector.tensor_copy(
                            kT16[g][:, nb * 512:(nb + 1) * 512],
                            ps[:, nb * 512:(nb + 1) * 512],
                        )
                for jb in range(NJB):
                    ps = psB.tile([128, 512], F32, tag="psv")
                    for kc in range(KC):
                        nc.tensor.matmul(
                            ps[:], xT16[kc][:, jb * 128:(jb + 1) * 128], wv16[kc][:],
                            start=(kc == 0), stop=(kc == KC - 1),
                        )
                    nc.vector.tensor_copy(v16[:, jb, :], ps[:])
                for g in range(4):
                    ps = psB.tile([128, ROWS], F32, tag="psq")
                    for kc in range(KC):
                        nc.tensor.matmul(
                            ps[:], wq16[kc][:, g * 128:(g + 1) * 128], xTq16[kc][:],
                            start=(kc == 0), stop=(kc == KC - 1),
                        )
                    nc.vector.tensor_copy(qT16[g][:], ps[:])

            # ---------------- Phase C: attention ----------------
            with (
                tc.tile_pool(name="psSim", bufs=1, space=bass.MemorySpace.PSUM) as psS,
                tc.tile_pool(name="psTr", bufs=2, space=bass.MemorySpace.PSUM) as psT,
                tc.tile_pool(name="psAv", bufs=2, space=bass.MemorySpace.PSUM) as psA,
            ):
                for sg in range(4):
                    # 8 row-tiles in lockstep: t = sub*NT + i, head = 2*sg+sub
                    ntile = 2 * NT
                    st = {
                        nm: stp.tile([128, ntile], F32, tag=nm, name=nm)
                        for nm in ("mx", "tau", "S", "cnt", "o", "oprev",
                                   "fprev", "fcur", "rc", "den", "dod",
                                   "step", "oneg")
                    }
                    rts = []
                    for t in range(ntile):
                        sub, i = t // NT, t % NT
                        ps = psS.tile([128, N], F32, tag="sim")
                        for nb in range(NNB):
                            nc.tensor.matmul(
                                ps[:, nb * 512:(nb + 1) * 512],
                                qT16[sg][sub * 64:(sub + 1) * 64,
                                         i * 128:(i + 1) * 128],
                                kT16[sg][sub * 64:(sub + 1) * 64,
                                         nb * 512:(nb + 1) * 512],
                                start=True, stop=True,
                            )
                        z = zp.tile([128, N], F16, tag="z")
                        # Relu copy keeps ACT on one LUT set; clamping z at 0
                        # is exact here since tau0 = rowmax-1 > 0 always
                        nc.scalar.activation(z[:], ps[:], AF.Relu)
                        nc.vector.reduce_max(st["mx"][:, t:t + 1], z[:], axis=AX.X)
                        # per-tile tau so z slots release without a barrier
                        nc.vector.tensor_scalar(
                            st["tau"][:, t:t + 1], st["mx"][:, t:t + 1],
                            -1.0, None, A.add)
                        rts.append((z, sub, i))

                    r16 = []
                    for t in range(ntile):
                        z = rts[t][0]
                        r = rp.tile([128, N], F16, tag="r")
                        nc.vector.tensor_scalar(
                            r[:], z[:], st["tau"][:, t:t + 1], 0.0,
                            A.subtract, A.max)
                        nc.vector.reduce_sum(st["S"][:, t:t + 1], r[:], axis=AX.X)
                        msk = scp.tile([128, N], F16, tag="scr")
                        nc.gpsimd.tensor_scalar(msk[:], r[:], 0.0, None, A.is_gt)
                        nc.vector.reduce_sum(st["cnt"][:, t:t + 1], msk[:], axis=AX.X)
                        r16.append(r)

                    # Newton step 1 (o relative to tau0): o = (S-1)/cnt
                    nc.vector.tensor_scalar(st["fprev"][:], st["S"][:], -1.0, None, A.add)
                    nc.vector.reciprocal(st["rc"][:], st["cnt"][:])
                    nc.vector.tensor_tensor(st["o"][:], st["fprev"][:], st["rc"][:], A.mult)
                    nc.vector.memset(st["oprev"][:], 0.0)
                    nc.vector.tensor_scalar(st["oneg"][:], st["o"][:], -1.0, None, A.mult)

                    for it in range(N_ITERS):
                        eng = F_ENGINES[it + 1]
                        for t in range(ntile):
                            r = r16[t]
                            if eng == "A":
                                scr = scp.tile([128, N], F16, tag="scr")
                                nc.scalar.activation(
                                    scr[:], r[:], AF.Relu,
                                    bias=st["oneg"][:, t:t + 1],
                                    accum_out=st["S"][:, t:t + 1])
                            else:
                                e = nc.vector if eng == "D" else nc.gpsimd
                                scr = scp.tile([128, N], F16, tag="scr")
                                e.tensor_scalar(
                                    scr[:], r[:], st["o"][:, t:t + 1], 0.0,
                                    A.subtract, A.max)
                                nc.vector.reduce_sum(
                                    st["S"][:, t:t + 1], scr[:], axis=AX.X)
                        # secant update; gate kills the step when the
                        # denominator is non-positive (fp16 noise near
                        # convergence) — without it o diverges to inf/NaN
                        nc.vector.tensor_scalar(st["fcur"][:], st["S"][:], -1.0, None, A.add)
                        nc.vector.tensor_tensor(st["den"][:], st["fprev"][:], st["fcur"][:], A.subtract)
                        gate = st["tau"]  # tau is dead after iter1; reuse
                        nc.vector.tensor_scalar(gate[:], st["den"][:], 1e-9, None, A.is_gt)
                        nc.vector.tensor_scalar(st["den"][:], st["den"][:], 1e-9, None, A.max)
                        nc.vector.reciprocal(st["rc"][:], st["den"][:])
                        nc.vector.tensor_tensor(st["dod"][:], st["o"][:], st["oprev"][:], A.subtract)
                        nc.vector.tensor_tensor(st["step"][:], st["fcur"][:], st["dod"][:], A.mult)
                        nc.vector.tensor_tensor(st["step"][:], st["step"][:], st["rc"][:], A.mult)
                        nc.vector.tensor_tensor(st["step"][:], st["step"][:], gate[:], A.mult)
                        nc.vector.tensor_copy(st["oprev"][:], st["o"][:])
                        nc.vector.tensor_copy(st["fprev"][:], st["fcur"][:])
                        nc.vector.tensor_tensor(st["o"][:], st["o"][:], st["step"][:], A.add)
                        nc.vector.tensor_scalar(st["oneg"][:], st["o"][:], -1.0, None, A.mult)

                    # final attn = relu(r - o), in place
                    for t in range(ntile):
                        r = r16[t]
                        nc.vector.tensor_scalar(
                            r[:], r[:], st["o"][:, t:t + 1], 0.0,
                            A.subtract, A.max)

                    # transpose + attn @ v per head
                    for sub in range(2):
                        h = 2 * sg + sub
                        aT = atp.tile([128, NJB, 512], F16, tag="aT")
                        for jc in range(NJB):
                            pt = psT.tile([128, 512], F16, tag="tr")
                            for i in range(NT):
                                nc.tensor.transpose(
                                    pt[:, i * 128:(i + 1) * 128],
                                    r16[sub * NT + i][:, jc * 128:(jc + 1) * 128],
                                    idn16[:],
                                )
                            if jc % 2 == 0:
                                nc.scalar.activation(aT[:, jc, :], pt[:], AF.Relu)
                            else:
                                nc.vector.tensor_copy(aT[:, jc, :], pt[:])
                        pav = psA.tile([64, 512], F32, tag="av")
                        for jc in range(NJB):
                            nc.tensor.matmul(
                                pav[:], v16[:, jc, h * 64:(h + 1) * 64],
                                aT[:, jc, :],
                                start=(jc == 0), stop=(jc == NJB - 1),
                            )
                        nc.vector.tensor_copy(
                            aoT[sg][sub * 64:(sub + 1) * 64, :], pav[:])

            # ---------------- Phase D: output projection ----------------
            with tc.tile_pool(name="psD", bufs=2, space=bass.MemorySpace.PSUM) as psD:
                for rb in range(NT):
                    ps = psD.tile([128, 512], F32, tag="pso")
                    for g in range(4):
                        nc.tensor.matmul(
                            ps[:], aoT[g][:, rb * 128:(rb + 1) * 128], wo16[g][:],
                            start=(g == 0), stop=(g == 3),
                        )
                    ob = op_.tile([128, 512], F32, tag="ob")
                    nc.vector.tensor_tensor(ob[:], ps[:], bias32[:], A.add)
                    nc.gpsimd.dma_start(out_ext[rb * 128:(rb + 1) * 128, :], ob[:])

    nc.compile()
    return nc


_NC_CACHE = None


def _get_nc():
    global _NC_CACHE
    if _NC_CACHE is None:
        _NC_CACHE = build()
    return _NC_CACHE


def make_in_maps(x, W_qkv, W_out, b_out):
    wq = np.ascontiguousarray(W_qkv[:, :512] * SCALE, dtype=np.float32)
    wk = np.ascontiguousarray(W_qkv[:, 512:1024], dtype=np.float32)
    wv = np.ascontiguousarray(W_qkv[:, 1024:1536], dtype=np.float32)
    wo = np.ascontiguousarray(W_out, dtype=np.float32)
    bias = np.ascontiguousarray(np.tile(b_out[None, :], (128, 1)), dtype=np.float32)
    idn = np.eye(128, dtype=np.float16)
    in_maps = []
    for c in range(8):
        b, r0 = c // 4, (c % 4) * ROWS
        xT = np.ascontiguousarray(x[b].T, dtype=np.float32)
        in_maps.append({
            "xT": xT,
            "xTq": np.ascontiguousarray(xT[:, r0:r0 + ROWS]),
            "wq": wq, "wk": wk, "wv": wv, "wo": wo,
            "bias": bias, "idn": idn,
        })
    return in_maps


def kernel(x, W_qkv, W_out, b_out, _trace=False, _results_box=None):
    nc = _get_nc()
    in_maps = make_in_maps(x, W_qkv, W_out, b_out)
    res = run_bass_kernel_spmd(nc, in_maps, list(range(8)), trace=_trace)
    if _results_box is not None:
        _results_box.append(res)
    out = np.zeros((B, N, D), np.float32)
    for c in range(8):
        b, r0 = c // 4, (c % 4) * ROWS
        out[b, r0:r0 + ROWS, :] = res.results[c]["out"]
    return out


# revision 7
# speedup vs baseline: 4.0601x; 4.0601x over previous
"""Sparse attention (sparsemax) TRN2 kernel — 8 NeuronCores, SPMD.

Sharding: i-row parallel. Core c handles batch b=c//4, query rows
[(c%4)*512, (c%4+1)*512) for ALL 8 heads. k/v projections are computed
for the full sequence on every core of a batch (cheap, PE has slack);
q projection only for the core's rows. No collectives — each core
produces complete output rows.

Sparsemax per row solved via threshold iteration instead of sort:
tau* solves sum(relu(z - tau)) = 1 (piecewise-linear decreasing in
tau). Init tau0 = rowmax - 1 (a guaranteed lower bound of tau*), one
Newton step using the support count, then 6 secant iterations (exact
once the support stabilizes; converges to the fp16 noise floor).
All iterations operate on r = relu(z - tau0) — identical f values,
lets z be clamped at 0 so every ACT pass can use the Relu table.

Compute dtype fp16 (matmuls + sparsemax tensors), f32 PSUM/stats.
attn@v needs attn^T: PE transpose via identity (DMA transpose hits
per-instruction semaphore-wait caps under Tile).
"""
import sys

sys.path.insert(0, "/opt/trn_rl_repo")

import numpy as np
import concourse.bass as bass
import concourse.bacc as bacc
import concourse.mybir as mybir
import concourse.tile as tile
from concourse.bass_utils import run_bass_kernel_spmd

F32 = mybir.dt.float32
F16 = mybir.dt.float16
A = mybir.AluOpType
AF = mybir.ActivationFunctionType
AX = mybir.AxisListType

B, N, D = 2, 2048, 512
H, DH = 8, 64
SCALE = DH ** -0.5
ROWS = 512          # query rows per core
NT = ROWS // 128    # 4 row tiles per head on this core
NJB = N // 128      # 16 key blocks
NNB = N // 512      # 4 matmul N-chunks over keys
KC = D // 128       # 4 contraction chunks over model dim

# f-eval engine per iteration round: iter1 + 6 secant.  D=DVE, A=ACT
# (GPS wide ops are 31us/pass — software path — never use them wide)
F_ENGINES = ["D", "A", "D", "A", "D", "A", "D"]
N_ITERS = len(F_ENGINES) - 1  # secant rounds after iter1


def build():
    nc = bacc.Bacc(None, target_bir_lowering=False)

    xT_ext = nc.declare_dram_parameter("xT", [D, N], F32, isOutput=False)
    xTq_ext = nc.declare_dram_parameter("xTq", [D, ROWS], F32, isOutput=False)
    wq_ext = nc.declare_dram_parameter("wq", [D, 512], F32, isOutput=False)
    wk_ext = nc.declare_dram_parameter("wk", [D, 512], F32, isOutput=False)
    wv_ext = nc.declare_dram_parameter("wv", [D, 512], F32, isOutput=False)
    wo_ext = nc.declare_dram_parameter("wo", [D, 512], F32, isOutput=False)
    bias_ext = nc.declare_dram_parameter("bias", [128, 512], F32, isOutput=False)
    idn_ext = nc.declare_dram_parameter("idn", [128, 128], F16, isOutput=False)
    out_ext = nc.declare_dram_parameter("out", [ROWS, 512], F32, isOutput=True)

    with tile.TileContext(nc) as tc:
        with (
            tc.tile_pool(name="persist", bufs=1) as pp,
            tc.tile_pool(name="stage", bufs=3) as stg,
            tc.tile_pool(name="rpool", bufs=8) as rp,
            tc.tile_pool(name="scrpool", bufs=4) as scp,
            tc.tile_pool(name="statpool", bufs=2) as stp,
            tc.tile_pool(name="attnT", bufs=2) as atp,
            tc.tile_pool(name="outp", bufs=2) as op_,
        ):
            # ---------------- Phase A: loads + fp16 casts ----------------
            def load_cast(ext, kc, width, tag):
                s = stg.tile([128, width], F32, tag=f"st{width}")
                nc.gpsimd.dma_start(s[:], ext[kc * 128:(kc + 1) * 128, :])
                t = pp.tile([128, width], F16, tag=f"{tag}{kc}", name=f"{tag}{kc}")
                nc.vector.tensor_copy(t[:], s[:])
                return t

            xT16 = [load_cast(xT_ext, kc, N, "xT") for kc in range(KC)]
            xTq16 = [load_cast(xTq_ext, kc, ROWS, "xTq") for kc in range(KC)]
            wq16 = [load_cast(wq_ext, kc, 512, "wq") for kc in range(KC)]
            wk16 = [load_cast(wk_ext, kc, 512, "wk") for kc in range(KC)]
            wv16 = [load_cast(wv_ext, kc, 512, "wv") for kc in range(KC)]
            wo16 = [load_cast(wo_ext, kc, 512, "wo") for kc in range(KC)]
            bias32 = pp.tile([128, 512], F32, tag="bias")
            nc.gpsimd.dma_start(bias32[:], bias_ext[:])
            idn16 = pp.tile([128, 128], F16, tag="idn")
            nc.gpsimd.dma_start(idn16[:], idn_ext[:])
            zeros16 = pp.tile([128, N], F16, tag="zeros16")
            nc.vector.memset(zeros16[:], 0.0)
            ones16 = pp.tile([128, N], F16, tag="ones16")
            nc.vector.memset(ones16[:], 1.0)

            # ---------------- Phase B: projections (PE) ----------------
            # kT[g]: [128(2 heads x 64), N] = (wk 2-head slice)^T @ x^T
            kT16 = [pp.tile([128, N], F16, tag=f"kT{g}", name=f"kT{g}") for g in range(4)]
            qT16 = [pp.tile([128, ROWS], F16, tag=f"qT{g}", name=f"qT{g}") for g in range(4)]
            v16 = pp.tile([128, NJB, 512], F16, tag="v16")
            aoT = [pp.tile([128, 512], F16, tag=f"aoT{g}", name=f"aoT{g}") for g in range(4)]

            with tc.tile_pool(name="psB", bufs=2, space=bass.MemorySpace.PSUM) as psB:
                for g in range(4):
                    ps = psB.tile([128, N], F32, tag="pskT", bufs=1)
                    for nb in range(NNB):
                        for kc in range(KC):
                            nc.tensor.matmul(
                                ps[:, nb * 512:(nb + 1) * 512],
                                wk16[kc][:, g * 128:(g + 1) * 128],
                                xT16[kc][:, nb * 512:(nb + 1) * 512],
                                start=(kc == 0), stop=(kc == KC - 1),
                            )
                    for nb in range(NNB):
                        nc.vector.tensor_copy(
                            kT16[g][:, nb * 512:(nb + 1) * 512],
                            ps[:, nb * 512:(nb + 1) * 512],
                        )
                for jb in range(NJB):
                    ps = psB.tile([128, 512], F32, tag="psv")
                    for kc in range(KC):
                        nc.tensor.matmul(
                            ps[:], xT16[kc][:, jb * 128:(jb + 1) * 128], wv16[kc][:],
                            start=(kc == 0), stop=(kc == KC - 1),
                        )
                    nc.vector.tensor_copy(v16[:, jb, :], ps[:])
                for g in range(4):
                    ps = psB.tile([128, ROWS], F32, tag="psq")
                    for kc in range(KC):
                        nc.tensor.matmul(
                            ps[:], wq16[kc][:, g * 128:(g + 1) * 128], xTq16[kc][:],
                            start=(kc == 0), stop=(kc == KC - 1),
                        )
                    nc.vector.tensor_copy(qT16[g][:], ps[:])

            # ---------------- Phase C: attention ----------------
            with (
                tc.tile_pool(name="psSim", bufs=1, space=bass.MemorySpace.PSUM) as psS,
                tc.tile_pool(name="psTr", bufs=2, space=bass.MemorySpace.PSUM) as psT,
                tc.tile_pool(name="psAv", bufs=2, space=bass.MemorySpace.PSUM) as psA,
            ):
                for sg in range(4):
                    # 8 row-tiles in lockstep: t = sub*NT + i, head = 2*sg+sub
                    ntile = 2 * NT
                    st = {
                        nm: stp.tile([128, ntile], F32, tag=nm, name=nm)
                        for nm in ("mx", "tau", "S", "cnt", "o", "oprev",
                                   "fprev", "fcur", "rc", "den", "dod",
                                   "step", "oneg")
                    }
                    rts = []
                    for t in range(ntile):
                        sub, i = t // NT, t % NT
                        ps = psS.tile([128, N], F32, tag="sim")
                        for nb in range(NNB):
                            nc.tensor.matmul(
                                ps[:, nb * 512:(nb + 1) * 512],
                                qT16[sg][sub * 64:(sub + 1) * 64,
                                         i * 128:(i + 1) * 128],
                                kT16[sg][sub * 64:(sub + 1) * 64,
                                         nb * 512:(nb + 1) * 512],
                                start=True, stop=True,
                            )
                        nc.vector.reduce_max(st["mx"][:, t:t + 1], ps[:], axis=AX.X)
                        # per-tile tau so psum slots release without a barrier
                        nc.vector.tensor_scalar(
                            st["tau"][:, t:t + 1], st["mx"][:, t:t + 1],
                            -1.0, None, A.add)
                        rts.append((ps, sub, i))

                    r16 = []
                    for t in range(ntile):
                        ps = rts[t][0]
                        r = rp.tile([128, N], F16, tag="r")
                        nc.vector.scalar_tensor_tensor(
                            r[:], ps[:], st["tau"][:, t:t + 1], zeros16[:],
                            op0=A.subtract, op1=A.max,
                            accum_out=st["S"][:, t:t + 1])
                        msk = scp.tile([128, N], F16, tag="scr")
                        nc.vector.scalar_tensor_tensor(
                            msk[:], r[:], 0.0, ones16[:],
                            op0=A.is_gt, op1=A.mult,
                            accum_out=st["cnt"][:, t:t + 1])
                        r16.append(r)

                    # Newton step 1 (o relative to tau0): o = (S-1)/cnt
                    nc.vector.tensor_scalar(st["fprev"][:], st["S"][:], -1.0, None, A.add)
                    nc.vector.reciprocal(st["rc"][:], st["cnt"][:])
                    nc.vector.tensor_tensor(st["o"][:], st["fprev"][:], st["rc"][:], A.mult)
                    nc.vector.memset(st["oprev"][:], 0.0)
                    nc.vector.tensor_scalar(st["oneg"][:], st["o"][:], -1.0, None, A.mult)

                    for it in range(N_ITERS):
                        eng = F_ENGINES[it + 1]
                        for t in range(ntile):
                            r = r16[t]
                            scr = scp.tile([128, N], F16, tag="scr")
                            if eng == "A":
                                nc.scalar.activation(
                                    scr[:], r[:], AF.Relu,
                                    bias=st["oneg"][:, t:t + 1],
                                    accum_out=st["S"][:, t:t + 1])
                            else:
                                nc.vector.scalar_tensor_tensor(
                                    scr[:], r[:], st["o"][:, t:t + 1], zeros16[:],
                                    op0=A.subtract, op1=A.max,
                                    accum_out=st["S"][:, t:t + 1])
                        # secant update; gate kills the step when the
                        # denominator is non-positive (fp16 noise near
                        # convergence) — without it o diverges to inf/NaN
                        nc.vector.tensor_scalar(st["fcur"][:], st["S"][:], -1.0, None, A.add)
                        nc.vector.tensor_tensor(st["den"][:], st["fprev"][:], st["fcur"][:], A.subtract)
                        gate = st["tau"]  # tau is dead after iter1; reuse
                        nc.vector.tensor_scalar(gate[:], st["den"][:], 1e-9, None, A.is_gt)
                        nc.vector.tensor_scalar(st["den"][:], st["den"][:], 1e-9, None, A.max)
                        nc.vector.reciprocal(st["rc"][:], st["den"][:])
                        nc.vector.tensor_tensor(st["dod"][:], st["o"][:], st["oprev"][:], A.subtract)
                        nc.vector.tensor_tensor(st["step"][:], st["fcur"][:], st["dod"][:], A.mult)
                        nc.vector.tensor_tensor(st["step"][:], st["step"][:], st["rc"][:], A.mult)
                        nc.vector.tensor_tensor(st["step"][:], st["step"][:], gate[:], A.mult)
                        nc.vector.tensor_copy(st["oprev"][:], st["o"][:])
                        nc.vector.tensor_copy(st["fprev"][:], st["fcur"][:])
                        nc.vector.tensor_tensor(st["o"][:], st["o"][:], st["step"][:], A.add)
                        nc.vector.tensor_scalar(st["oneg"][:], st["o"][:], -1.0, None, A.mult)

                    # final attn = relu(r - o), in place
                    for t in range(ntile):
                        r = r16[t]
                        nc.vector.tensor_scalar(
                            r[:], r[:], st["o"][:, t:t + 1], 0.0,
                            A.subtract, A.max)

                    # transpose + attn @ v per head
                    for sub in range(2):
                        h = 2 * sg + sub
                        aT = atp.tile([128, NJB, 512], F16, tag="aT")
                        for jc in range(NJB):
                            pt = psT.tile([128, 512], F16, tag="tr")
                            for i in range(NT):
                                nc.tensor.transpose(
                                    pt[:, i * 128:(i + 1) * 128],
                                    r16[sub * NT + i][:, jc * 128:(jc + 1) * 128],
                                    idn16[:],
                                )
                            if jc % 2 == 0:
                                nc.scalar.activation(aT[:, jc, :], pt[:], AF.Relu)
                            else:
                                nc.vector.tensor_copy(aT[:, jc, :], pt[:])
                        pav = psA.tile([64, 512], F32, tag="av")
                        for jc in range(NJB):
                            nc.tensor.matmul(
                                pav[:], v16[:, jc, h * 64:(h + 1) * 64],
                                aT[:, jc, :],
                                start=(jc == 0), stop=(jc == NJB - 1),
                            )
                        nc.vector.tensor_copy(
                            aoT[sg][sub * 64:(sub + 1) * 64, :], pav[:])

            # ---------------- Phase D: output projection ----------------
            with tc.tile_pool(name="psD", bufs=2, space=bass.MemorySpace.PSUM) as psD:
                for rb in range(NT):
                    ps = psD.tile([128, 512], F32, tag="pso")
                    for g in range(4):
                        nc.tensor.matmul(
                            ps[:], aoT[g][:, rb * 128:(rb + 1) * 128], wo16[g][:],
                            start=(g == 0), stop=(g == 3),
                        )
                    ob = op_.tile([128, 512], F32, tag="ob")
                    nc.vector.tensor_tensor(ob[:], ps[:], bias32[:], A.add)
                    nc.gpsimd.dma_start(out_ext[rb * 128:(rb + 1) * 128, :], ob[:])

    nc.compile()
    return nc


_NC_CACHE = None


def _get_nc():
    global _NC_CACHE
    if _NC_CACHE is None:
        _NC_CACHE = build()
    return _NC_CACHE


def make_in_maps(x, W_qkv, W_out, b_out):
    wq = np.ascontiguousarray(W_qkv[:, :512] * SCALE, dtype=np.float32)
    wk = np.ascontiguousarray(W_qkv[:, 512:1024], dtype=np.float32)
    wv = np.ascontiguousarray(W_qkv[:, 1024:1536], dtype=np.float32)
    wo = np.ascontiguousarray(W_out, dtype=np.float32)
    bias = np.ascontiguousarray(np.tile(b_out[None, :], (128, 1)), dtype=np.float32)
    idn = np.eye(128, dtype=np.float16)
    in_maps = []
    for c in range(8):
        b, r0 = c // 4, (c % 4) * ROWS
        xT = np.ascontiguousarray(x[b].T, dtype=np.float32)
        in_maps.append({
            "xT": xT,
            "xTq": np.ascontiguousarray(xT[:, r0:r0 + ROWS]),
            "wq": wq, "wk": wk, "wv": wv, "wo": wo,
            "bias": bias, "idn": idn,
        })
    return in_maps


def kernel(x, W_qkv, W_out, b_out, _trace=False, _results_box=None):
    nc = _get_nc()
    in_maps = make_in_maps(x, W_qkv, W_out, b_out)
    res = run_bass_kernel_spmd(nc, in_maps, list(range(8)), trace=_trace)
    if _results_box is not None:
        _results_box.append(res)
    out = np.zeros((B, N, D), np.float32)
    for c in range(8):
        b, r0 = c // 4, (c % 4) * ROWS
        out[b, r0:r0 + ROWS, :] = res.results[c]["out"]
    return out


# revision 8
# speedup vs baseline: 5.2619x; 1.2960x over previous
"""Sparse attention (sparsemax) TRN2 kernel — 8 NeuronCores, SPMD.

Sharding: i-row parallel. Core c handles batch b=c//4, query rows
[(c%4)*512, (c%4+1)*512) for ALL 8 heads. k/v projections are computed
for the full sequence on every core of a batch (cheap, PE has slack);
q projection only for the core's rows. No collectives — each core
produces complete output rows.

Sparsemax per row solved via threshold iteration instead of sort:
tau* solves sum(relu(z - tau)) = 1 (piecewise-linear decreasing in
tau). Init tau0 = rowmax - 1 (a guaranteed lower bound of tau*), one
Newton step using the support count, then 6 secant iterations (exact
once the support stabilizes; converges to the fp16 noise floor).
All iterations operate on r = relu(z - tau0) — identical f values,
lets z be clamped at 0 so every ACT pass can use the Relu table.

Compute dtype fp16 (matmuls + sparsemax tensors), f32 PSUM/stats.
attn@v needs attn^T: PE transpose via identity (DMA transpose hits
per-instruction semaphore-wait caps under Tile).
"""
import sys

sys.path.insert(0, "/opt/trn_rl_repo")

import numpy as np
import concourse.bass as bass
import concourse.bacc as bacc
import concourse.mybir as mybir
import concourse.tile as tile
from concourse.bass_utils import run_bass_kernel_spmd

F32 = mybir.dt.float32
F16 = mybir.dt.float16
A = mybir.AluOpType
AF = mybir.ActivationFunctionType
AX = mybir.AxisListType

B, N, D = 2, 2048, 512
H, DH = 8, 64
SCALE = DH ** -0.5
ROWS = 512          # query rows per core
NT = ROWS // 128    # 4 row tiles per head on this core
NJB = N // 128      # 16 key blocks
NNB = N // 512      # 4 matmul N-chunks over keys
KC = D // 128       # 4 contraction chunks over model dim

# Wide passes cost ~2.3us on DVE (stt, 1x) and ~2.0us on ACT — split each
# round's 8 tiles across BOTH engines by parity so they run concurrently.
N_ITERS = 6  # secant rounds after the first Newton step


def build():
    nc = bacc.Bacc(None, target_bir_lowering=False)

    xT_ext = nc.declare_dram_parameter("xT", [D, N], F32, isOutput=False)
    xTq_ext = nc.declare_dram_parameter("xTq", [D, ROWS], F32, isOutput=False)
    wq_ext = nc.declare_dram_parameter("wq", [D, 512], F32, isOutput=False)
    wk_ext = nc.declare_dram_parameter("wk", [D, 512], F32, isOutput=False)
    wv_ext = nc.declare_dram_parameter("wv", [D, 512], F32, isOutput=False)
    wo_ext = nc.declare_dram_parameter("wo", [D, 512], F32, isOutput=False)
    bias_ext = nc.declare_dram_parameter("bias", [128, 512], F32, isOutput=False)
    idn_ext = nc.declare_dram_parameter("idn", [128, 128], F16, isOutput=False)
    out_ext = nc.declare_dram_parameter("out", [ROWS, 512], F32, isOutput=True)

    with tile.TileContext(nc) as tc:
        with (
            tc.tile_pool(name="persist", bufs=1) as pp,
            tc.tile_pool(name="stage", bufs=2) as stg,
            tc.tile_pool(name="rpool", bufs=12) as rp,
            tc.tile_pool(name="scrpool", bufs=3) as scp,
            tc.tile_pool(name="statpool", bufs=2) as stp,
            tc.tile_pool(name="attnT", bufs=2) as atp,
            tc.tile_pool(name="outp", bufs=2) as op_,
        ):
            # ---------------- Phase A: loads + fp16 casts ----------------
            def load_cast(ext, kc, width, tag):
                s = stg.tile([128, width], F32, tag=f"st{width}")
                nc.gpsimd.dma_start(s[:], ext[kc * 128:(kc + 1) * 128, :])
                t = pp.tile([128, width], F16, tag=f"{tag}{kc}", name=f"{tag}{kc}")
                nc.vector.tensor_copy(t[:], s[:])
                return t

            xT16 = [load_cast(xT_ext, kc, N, "xT") for kc in range(KC)]
            xTq16 = [load_cast(xTq_ext, kc, ROWS, "xTq") for kc in range(KC)]
            wq16 = [load_cast(wq_ext, kc, 512, "wq") for kc in range(KC)]
            wk16 = [load_cast(wk_ext, kc, 512, "wk") for kc in range(KC)]
            wv16 = [load_cast(wv_ext, kc, 512, "wv") for kc in range(KC)]
            wo16 = [load_cast(wo_ext, kc, 512, "wo") for kc in range(KC)]
            bias32 = pp.tile([128, 512], F32, tag="bias")
            nc.gpsimd.dma_start(bias32[:], bias_ext[:])
            idn16 = pp.tile([128, 128], F16, tag="idn")
            nc.gpsimd.dma_start(idn16[:], idn_ext[:])
            zeros16 = pp.tile([128, N], F16, tag="zeros16")
            nc.vector.memset(zeros16[:], 0.0)
            ones16 = pp.tile([128, N], F16, tag="ones16")
            nc.vector.memset(ones16[:], 1.0)

            # ---------------- Phase B: projections (PE) ----------------
            # kT[g]: [128(2 heads x 64), N] = (wk 2-head slice)^T @ x^T
            kT16 = [pp.tile([128, N], F16, tag=f"kT{g}", name=f"kT{g}") for g in range(4)]
            qT16 = [pp.tile([128, ROWS], F16, tag=f"qT{g}", name=f"qT{g}") for g in range(4)]
            v16 = pp.tile([128, NJB, 512], F16, tag="v16")
            aoT = [pp.tile([128, 512], F16, tag=f"aoT{g}", name=f"aoT{g}") for g in range(4)]

            with tc.tile_pool(name="psB", bufs=2, space=bass.MemorySpace.PSUM) as psB:
                for g in range(4):
                    ps = psB.tile([128, N], F32, tag="pskT", bufs=1)
                    for nb in range(NNB):
                        for kc in range(KC):
                            nc.tensor.matmul(
                                ps[:, nb * 512:(nb + 1) * 512],
                                wk16[kc][:, g * 128:(g + 1) * 128],
                                xT16[kc][:, nb * 512:(nb + 1) * 512],
                                start=(kc == 0), stop=(kc == KC - 1),
                            )
                    for nb in range(NNB):
                        nc.vector.tensor_copy(
                            kT16[g][:, nb * 512:(nb + 1) * 512],
                            ps[:, nb * 512:(nb + 1) * 512],
                        )
                for jb in range(NJB):
                    ps = psB.tile([128, 512], F32, tag="psv")
                    for kc in range(KC):
                        nc.tensor.matmul(
                            ps[:], xT16[kc][:, jb * 128:(jb + 1) * 128], wv16[kc][:],
                            start=(kc == 0), stop=(kc == KC - 1),
                        )
                    nc.vector.tensor_copy(v16[:, jb, :], ps[:])
                for g in range(4):
                    ps = psB.tile([128, ROWS], F32, tag="psq")
                    for kc in range(KC):
                        nc.tensor.matmul(
                            ps[:], wq16[kc][:, g * 128:(g + 1) * 128], xTq16[kc][:],
                            start=(kc == 0), stop=(kc == KC - 1),
                        )
                    nc.vector.tensor_copy(qT16[g][:], ps[:])

            # ---------------- Phase C: attention ----------------
            with (
                tc.tile_pool(name="psSim", bufs=1, space=bass.MemorySpace.PSUM) as psS,
                tc.tile_pool(name="psTr", bufs=2, space=bass.MemorySpace.PSUM) as psT,
                tc.tile_pool(name="psAv", bufs=2, space=bass.MemorySpace.PSUM) as psA,
            ):
                for sg in range(4):
                    # 8 row-tiles in lockstep: t = sub*NT + i, head = 2*sg+sub
                    ntile = 2 * NT
                    st = {
                        nm: stp.tile([128, ntile], F32, tag=nm, name=nm)
                        for nm in ("mx", "tau", "tauneg", "S", "cnt", "o",
                                   "oprev", "fprev", "fcur", "rc", "den",
                                   "dod", "step", "oneg")
                    }
                    rts = []
                    for t in range(ntile):
                        sub, i = t // NT, t % NT
                        ps = psS.tile([128, N], F32, tag="sim")
                        for nb in range(NNB):
                            nc.tensor.matmul(
                                ps[:, nb * 512:(nb + 1) * 512],
                                qT16[sg][sub * 64:(sub + 1) * 64,
                                         i * 128:(i + 1) * 128],
                                kT16[sg][sub * 64:(sub + 1) * 64,
                                         nb * 512:(nb + 1) * 512],
                                start=True, stop=True,
                            )
                        nc.vector.reduce_max(st["mx"][:, t:t + 1], ps[:], axis=AX.X)
                        # per-tile tau so psum slots release without a barrier
                        nc.vector.tensor_scalar(
                            st["tau"][:, t:t + 1], st["mx"][:, t:t + 1],
                            -1.0, None, A.add)
                        nc.vector.tensor_scalar(
                            st["tauneg"][:, t:t + 1], st["mx"][:, t:t + 1],
                            -1.0, 1.0, A.mult, A.add)
                        rts.append((ps, sub, i))

                    r16 = []
                    for t in range(ntile):
                        ps = rts[t][0]
                        r = rp.tile([128, N], F16, tag="r")
                        if t % 2 == 0:
                            nc.vector.scalar_tensor_tensor(
                                r[:], ps[:], st["tau"][:, t:t + 1], zeros16[:],
                                op0=A.subtract, op1=A.max,
                                accum_out=st["S"][:, t:t + 1])
                        else:
                            nc.scalar.activation(
                                r[:], ps[:], AF.Relu,
                                bias=st["tauneg"][:, t:t + 1],
                                accum_out=st["S"][:, t:t + 1])
                        msk = scp.tile([128, N], F16, tag="scr")
                        nc.vector.scalar_tensor_tensor(
                            msk[:], r[:], 0.0, ones16[:],
                            op0=A.is_gt, op1=A.mult,
                            accum_out=st["cnt"][:, t:t + 1])
                        r16.append(r)

                    # Newton step 1 (o relative to tau0): o = (S-1)/cnt
                    nc.vector.tensor_scalar(st["fprev"][:], st["S"][:], -1.0, None, A.add)
                    nc.vector.reciprocal(st["rc"][:], st["cnt"][:])
                    nc.vector.tensor_tensor(st["o"][:], st["fprev"][:], st["rc"][:], A.mult)
                    nc.vector.memset(st["oprev"][:], 0.0)
                    nc.vector.tensor_scalar(st["oneg"][:], st["o"][:], -1.0, None, A.mult)

                    for it in range(N_ITERS):
                        for t in range(ntile):
                            eng = "D" if (it + t) % 2 == 0 else "A"
                            r = r16[t]
                            scr = scp.tile([128, N], F16, tag="scr")
                            if eng == "A":
                                nc.scalar.activation(
                                    scr[:], r[:], AF.Relu,
                                    bias=st["oneg"][:, t:t + 1],
                                    accum_out=st["S"][:, t:t + 1])
                            else:
                                nc.vector.scalar_tensor_tensor(
                                    scr[:], r[:], st["o"][:, t:t + 1], zeros16[:],
                                    op0=A.subtract, op1=A.max,
                                    accum_out=st["S"][:, t:t + 1])
                        # secant update; gate kills the step when the
                        # denominator is non-positive (fp16 noise near
                        # convergence) — without it o diverges to inf/NaN
                        nc.vector.tensor_scalar(st["fcur"][:], st["S"][:], -1.0, None, A.add)
                        nc.vector.tensor_tensor(st["den"][:], st["fprev"][:], st["fcur"][:], A.subtract)
                        gate = st["tau"]  # tau is dead after iter1; reuse
                        nc.vector.tensor_scalar(gate[:], st["den"][:], 1e-9, None, A.is_gt)
                        nc.vector.tensor_scalar(st["den"][:], st["den"][:], 1e-9, None, A.max)
                        nc.vector.reciprocal(st["rc"][:], st["den"][:])
                        nc.vector.tensor_tensor(st["dod"][:], st["o"][:], st["oprev"][:], A.subtract)
                        nc.vector.tensor_tensor(st["step"][:], st["fcur"][:], st["dod"][:], A.mult)
                        nc.vector.tensor_tensor(st["step"][:], st["step"][:], st["rc"][:], A.mult)
                        nc.vector.tensor_tensor(st["step"][:], st["step"][:], gate[:], A.mult)
                        nc.vector.tensor_copy(st["oprev"][:], st["o"][:])
                        nc.vector.tensor_copy(st["fprev"][:], st["fcur"][:])
                        nc.vector.tensor_tensor(st["o"][:], st["o"][:], st["step"][:], A.add)
                        nc.vector.tensor_scalar(st["oneg"][:], st["o"][:], -1.0, None, A.mult)

                    # final attn = relu(r - o), in place
                    for t in range(ntile):
                        r = r16[t]
                        nc.vector.tensor_scalar(
                            r[:], r[:], st["o"][:, t:t + 1], 0.0,
                            A.subtract, A.max)

                    # transpose + attn @ v per head
                    for sub in range(2):
                        h = 2 * sg + sub
                        aT = atp.tile([128, NJB, 512], F16, tag="aT")
                        for jc in range(NJB):
                            pt = psT.tile([128, 512], F16, tag="tr")
                            for i in range(NT):
                                nc.tensor.transpose(
                                    pt[:, i * 128:(i + 1) * 128],
                                    r16[sub * NT + i][:, jc * 128:(jc + 1) * 128],
                                    idn16[:],
                                )
                            if jc % 2 == 0:
                                nc.scalar.activation(aT[:, jc, :], pt[:], AF.Relu)
                            else:
                                nc.vector.tensor_copy(aT[:, jc, :], pt[:])
                        pav = psA.tile([64, 512], F32, tag="av")
                        for jc in range(NJB):
                            nc.tensor.matmul(
                                pav[:], v16[:, jc, h * 64:(h + 1) * 64],
                                aT[:, jc, :],
                                start=(jc == 0), stop=(jc == NJB - 1),
                            )
                        nc.vector.tensor_copy(
                            aoT[sg][sub * 64:(sub + 1) * 64, :], pav[:])

            # ---------------- Phase D: output projection ----------------
            with tc.tile_pool(name="psD", bufs=2, space=bass.MemorySpace.PSUM) as psD:
                for rb in range(NT):
                    ps = psD.tile([128, 512], F32, tag="pso")
                    for g in range(4):
                        nc.tensor.matmul(
                            ps[:], aoT[g][:, rb * 128:(rb + 1) * 128], wo16[g][:],
                            start=(g == 0), stop=(g == 3),
                        )
                    ob = op_.tile([128, 512], F32, tag="ob")
                    nc.vector.tensor_tensor(ob[:], ps[:], bias32[:], A.add)
                    nc.gpsimd.dma_start(out_ext[rb * 128:(rb + 1) * 128, :], ob[:])

    nc.compile()
    return nc


_NC_CACHE = None


def _get_nc():
    global _NC_CACHE
    if _NC_CACHE is None:
        _NC_CACHE = build()
    return _NC_CACHE


def make_in_maps(x, W_qkv, W_out, b_out):
    wq = np.ascontiguousarray(W_qkv[:, :512] * SCALE, dtype=np.float32)
    wk = np.ascontiguousarray(W_qkv[:, 512:1024], dtype=np.float32)
    wv = np.ascontiguousarray(W_qkv[:, 1024:1536], dtype=np.float32)
    wo = np.ascontiguousarray(W_out, dtype=np.float32)
    bias = np.ascontiguousarray(np.tile(b_out[None, :], (128, 1)), dtype=np.float32)
    idn = np.eye(128, dtype=np.float16)
    in_maps = []
    for c in range(8):
        b, r0 = c // 4, (c % 4) * ROWS
        xT = np.ascontiguousarray(x[b].T, dtype=np.float32)
        in_maps.append({
            "xT": xT,
            "xTq": np.ascontiguousarray(xT[:, r0:r0 + ROWS]),
            "wq": wq, "wk": wk, "wv": wv, "wo": wo,
            "bias": bias, "idn": idn,
        })
    return in_maps


def kernel(x, W_qkv, W_out, b_out, _trace=False, _results_box=None):
    nc = _get_nc()
    in_maps = make_in_maps(x, W_qkv, W_out, b_out)
    res = run_bass_kernel_spmd(nc, in_maps, list(range(8)), trace=_trace)
    if _results_box is not None:
        _results_box.append(res)
    out = np.zeros((B, N, D), np.float32)
    for c in range(8):
        b, r0 = c // 4, (c % 4) * ROWS
        out[b, r0:r0 + ROWS, :] = res.results[c]["out"]
    return out
